# revision 6
# baseline (speedup 1.0000x reference)
"""DeformableConv2D (B=8, C=F=256, H=W=64, K=3x3) on 8 Trainium2 NeuronCores.

Sharding: data-parallel over batch - each of the 8 cores processes one sample.

Per-core pipeline (v7):
  1. offset/mask 3x3 SAME convs as shifted bf16 matmuls (f32 PSUM), output
     rows [dy(9) | dx(9) | pad | mask(9)@32]; conv split in two halves with
     the pixel-partition PE transposes interleaved into the second half.
  2. Bilinear pipeline in pixel-partition f32: fused floor/frac over the
     joint dy|dx block, mask sigmoid after the transpose, gather indices
     wrapped to the 16-partition dma_gather layout (log-doubling replicate).
  3. 36 corner-product planes q=(2*xc+yc)*9+k kept as PE-transposed
     wrapped-j rows (plrow[36, 4096] bf16).
  4. Main loop over (chunk=1024 px, tap): ONE dma_gather per unit fetches
     all 4 bilinear corners x 256 ch in a single 2KB elem (xg2 row-pair
     layout); this keeps the GPSIMD engine (the bottleneck) at its floor.
  5. Corner multiplies on DVE against planes replicated across partitions
     by selector-matmul PE broadcasts (PSUM -> bf16 via Act copies); one
     merged yc-add; the xc-sum is folded into the GEMM (x2 contraction).
  6. bf16 GEMM into f32 PSUM [128, 1024] x2; output written in wrapped-j
     column order, host reorders.

kernel(**inputs) takes the FULL batch and returns the FULL [8,256,64,64] f32
output.
"""

import dataclasses
from contextlib import ExitStack

import numpy as np

import concourse.bass as bass
import concourse.bacc as bacc
import concourse.tile as tile
from concourse import mybir
from concourse.bass_utils import run_bass_kernel_spmd

H = W = 64
HW = H * W
C = 256
F = 256
K = 9
OC = 41  # conv out rows: 0-8 dy, 9-17 dx, 32-40 mask
PAD = 8
HP = H + 2 * PAD  # 80
WP = W + 2 * PAD  # 80
NROW = HP * WP  # 6400
H1 = H + 2  # 66
W1 = W + 2
HW1 = H1 * W1  # 4356
MARG = 68

FP32 = mybir.dt.float32
I32 = mybir.dt.int32
BF16 = mybir.dt.bfloat16
I16 = mybir.dt.int16
AX = mybir.AluOpType
AF = mybir.ActivationFunctionType

CHUNK = 1024
NCHUNK = HW // CHUNK  # 4
Q = 4 * K  # 36 planes
NCORES = 8

# corners (xc, yc, k) routed to the PE/Act/DVE path instead of GPSIMD gatings.
# Pool is fully occupied by the gathers, so all corners are offloaded.
OFFLOAD = {(xc, yc, k) for k in range(K) for xc in range(2) for yc in range(2)}
# taps whose third add (s0 + s1) is folded into the GEMM (x2 contraction)
FOLD = set(range(K))


def host_inputs(x, w_offset, w_mask, w_deform):
    """Per-sample layout prep. x: [C,H,W] float32 one sample."""
    import ml_dtypes

    ins = {}
    xp1 = np.zeros((C, H1, W1), ml_dtypes.bfloat16)
    xp1[:, 1:-1, 1:-1] = x
    ins["xpad1"] = np.ascontiguousarray(xp1.reshape(C, HW1))

    # xg2 row (y, x) = [xpad[y, x, :], xpad[y+1, x, :]]  (bf16)
    xp2 = np.zeros((HP + 1, WP, C), ml_dtypes.bfloat16)
    xp2[PAD : PAD + H, PAD : PAD + W, :] = np.transpose(x, (1, 2, 0)).astype(
        ml_dtypes.bfloat16
    )
    xg2 = np.concatenate([xp2[:-1], xp2[1:]], axis=2)  # [HP, WP, 2C]
    ins["xg2"] = np.ascontiguousarray(xg2.reshape(NROW, 2 * C))

    # conv weights, out-channel order [dy(9) | dx(9) | pad | mask(9) at 32]
    wt = np.zeros((3, 3, C, OC), np.float32)
    wo = np.transpose(w_offset, (2, 3, 1, 0))  # [3,3,C,18]
    wt[:, :, :, 0:9] = wo[:, :, :, 0::2]  # dy_k = offset channel 2k
    wt[:, :, :, 9:18] = wo[:, :, :, 1::2]  # dx_k = offset channel 2k+1
    wt[:, :, :, 32:41] = np.transpose(w_mask, (2, 3, 1, 0))
    ins["wconv"] = np.ascontiguousarray(
        wt.reshape(K, 2, 128, OC), dtype=ml_dtypes.bfloat16
    )

    wd = np.transpose(w_deform.reshape(F, C, K), (2, 1, 0))  # [k, c, f]
    ins["wdef"] = np.ascontiguousarray(
        wd.reshape(K, 2, 128, F).astype(ml_dtypes.bfloat16)
    )

    p = np.arange(HW)
    hh = (p // W).astype(np.float32)
    ww = (p % W).astype(np.float32)
    ky = np.repeat(np.arange(3) - 1, 3).astype(np.float32)
    kx = np.tile(np.arange(3) - 1, 3).astype(np.float32)
    basey = (hh[:, None] + ky[None, :]).reshape(32, 128, K).transpose(1, 0, 2)
    basex = (ww[:, None] + kx[None, :]).reshape(32, 128, K).transpose(1, 0, 2)
    ins["basey"] = np.ascontiguousarray(basey, dtype=np.float32)
    ins["basex"] = np.ascontiguousarray(basex, dtype=np.float32)
    ins["ident"] = np.eye(128, dtype=np.float32)
    ins["ones2"] = np.ones((128, 2), np.float32)
    # sel[p, q, :] = (p == q): stationary that broadcasts plrow row q to all
    # 128 PSUM partitions
    sel = np.zeros((Q, Q, 128), ml_dtypes.bfloat16)
    for q in range(Q):
        sel[q, q, :] = 1.0
    ins["sel"] = sel.reshape(Q, Q * 128)
    return ins


def declare_inputs(nc):
    t = {}
    t["xpad1"] = nc.dram_tensor("xpad1", [C, HW1], BF16, kind="ExternalInput")
    t["xg2"] = nc.dram_tensor("xg2", [NROW, 2 * C], BF16, kind="ExternalInput")
    t["wconv"] = nc.dram_tensor("wconv", [K, 2, 128, OC], BF16, kind="ExternalInput")
    t["wdef"] = nc.dram_tensor("wdef", [K, 2, 128, F], BF16, kind="ExternalInput")
    t["basey"] = nc.dram_tensor("basey", [128, 32, K], FP32, kind="ExternalInput")
    t["basex"] = nc.dram_tensor("basex", [128, 32, K], FP32, kind="ExternalInput")
    t["ident"] = nc.dram_tensor("ident", [128, 128], FP32, kind="ExternalInput")
    t["ones2"] = nc.dram_tensor("ones2", [128, 2], FP32, kind="ExternalInput")
    t["sel"] = nc.dram_tensor("sel", [Q, Q * 128], BF16, kind="ExternalInput")
    # columns in wrapped-j order: j = 16*(32a + t) + b <-> pixel 128t + 16a + b
    t["out"] = nc.dram_tensor("out", [F, HW], FP32, kind="ExternalOutput")
    return t


def build(nc, tc, ctx: ExitStack, t):
    keep = ctx.enter_context(tc.tile_pool(name="keep", bufs=1))

    ident = keep.tile([128, 128], FP32)
    ones2 = keep.tile([128, 2], FP32)
    sel = keep.tile([Q, Q * 128], BF16)
    wdef_sb = keep.tile([128, K * 2 * F], BF16)
    widx = keep.tile([128, K, HW // 16], I16)
    plrow = keep.tile([Q, HW], BF16)  # wrapped-j order plane rows

    def load_aux():
        # off the critical path; issued on Act/DVE DMA queues
        nc.scalar.dma_start(ident[:], t["ident"].ap())
        nc.scalar.dma_start(ones2[:], t["ones2"].ap())
        nc.scalar.dma_start(sel[:], t["sel"].ap())
        nc.scalar.dma_start(
            wdef_sb[:].rearrange("p (k c f) -> p k c f", k=K, c=2),
            t["wdef"].ap().rearrange("k c p f -> p k c f"),
        )

    # ================= prologue =================
    with tc.tile_pool(name="prol", bufs=1) as prol, tc.tile_pool(
        name="stgp", bufs=4
    ) as stgp, tc.tile_pool(
        name="prps", bufs=2, space="PSUM"
    ) as prps, tc.tile_pool(name="trps", bufs=3, space="PSUM") as trps:
        wconv_sb = prol.tile([128, K * 2 * OC], BF16, tag="wconv")
        nc.sync.dma_start(
            wconv_sb[:].rearrange("p (k c o) -> p k c o", k=K, c=2),
            t["wconv"].ap().rearrange("k c p o -> p k c o"),
        )
        xp1 = [
            prol.tile([128, HW1 + 2 * MARG], BF16, tag=f"xp1_{i}", name=f"xp1_{i}")
            for i in range(2)
        ]
        for i in range(2):
            nc.vector.memset(xp1[i][:, 0:MARG], 0.0)
            nc.vector.memset(xp1[i][:, MARG + HW1 :], 0.0)
            nc.sync.dma_start(
                xp1[i][:, MARG : MARG + HW1], t["xpad1"].ap()[bass.ts(i, 128), :]
            )
        load_aux()

        # conv into two half tiles; pixT transposes interleaved with the
        # second half's matmuls so they overlap on all engines.
        # A: rows 0..39 (tcols 0..18); B: rows 40..65
        JSPLIT = 40 * W1  # 2640
        convA = prol.tile([OC, JSPLIT], FP32, tag="convA")
        convB = prol.tile([OC, HW1 - JSPLIT], FP32, tag="convB")
        NCONV = 4 * W1  # 264 (4 rows, 1 PSUM bank)
        wviews = wconv_sb[:].rearrange("p (k c o) -> p k c o", k=K, c=2)
        pixT = prol.tile([128, 32, OC], FP32, tag="pixT")

        def conv_row(h):  # [OC, W1] view of conv output row h
            if (h + 1) * W1 <= JSPLIT:
                return convA[:, h * W1 : (h + 1) * W1]
            return convB[:, h * W1 - JSPLIT : (h + 1) * W1 - JSPLIT]

        def emit_transpose(tcol):
            h0 = 2 * tcol
            stage = stgp.tile([OC, 128], FP32, tag="tr_stage", name=f"st{tcol}")
            for r in range(2):
                nc.vector.tensor_copy(
                    stage[:, 64 * r : 64 * r + 64],
                    conv_row(h0 + 1 + r)[:, 1 : 1 + W],
                )
            ps = trps.tile([128, OC], FP32, tag="tr_ps")
            nc.tensor.transpose(ps[:], stage[:], ident[:OC, :OC])
            if tcol % 2:
                nc.vector.tensor_copy(pixT[:, tcol, :], ps[:])
            else:
                nc.scalar.copy(pixT[:, tcol, :], ps[:])

        def emit_conv_block(j0):
            n = min(NCONV, HW1 - j0)
            ps = prps.tile([OC, NCONV], FP32, tag="conv_ps")
            first = True
            for ci in range(2):
                for k in range(K):
                    off = (k // 3 - 1) * W1 + (k % 3 - 1)
                    nc.tensor.matmul(
                        ps[:, :n],
                        wviews[:, k, ci, :],
                        xp1[ci][:, MARG + j0 + off : MARG + j0 + off + n],
                        start=first,
                        stop=(ci == 1 and k == K - 1),
                    )
                    first = False
            if j0 < JSPLIT:
                nc.scalar.copy(convA[:, j0 : j0 + n], ps[:, :n])
            else:
                nc.scalar.copy(convB[:, j0 - JSPLIT : j0 - JSPLIT + n], ps[:, :n])

        for j0 in range(0, JSPLIT, NCONV):
            emit_conv_block(j0)
        # second half: interleave pixT transposes as their rows become ready
        # (tcol T reads rows 2T+1, 2T+2; A covers rows < 40)
        pend = list(range(32))
        for j0 in range(JSPLIT, HW1, NCONV):
            emit_conv_block(j0)
            rows_done = min((j0 + NCONV), HW1) // W1
            while pend and 2 * pend[0] + 2 < rows_done:
                emit_transpose(pend.pop(0))
        for tcol in pend:
            emit_transpose(tcol)
        nc.scalar.activation(
            pixT[:, :, 32:41], pixT[:, :, 32:41], AF.Sigmoid
        )

        # ---- coefficient pipeline (f32, pixel-partition) ----
        # dy|dx processed together: pixT cols 0:18 match base2 = [basey|basex]
        def pt2(tag):
            return prol.tile([128, 32, 2 * K], FP32, tag=tag, name=tag)

        typ = pt2("typ")
        fyx = pt2("fyx")
        wyx = pt2("wyx")
        cr = pt2("cr")
        mwy0 = prol.tile([128, 32, K], FP32, tag="mwy0", name="mwy0")
        mwy1 = prol.tile([128, 32, K], FP32, tag="mwy1", name="mwy1")
        iy = prol.tile([128, 32, 2 * K], I32, tag="iy")
        base2 = prol.tile([128, 32, 2 * K], FP32, tag="base2")
        nc.sync.dma_start(base2[:, :, 0:9], t["basey"].ap())
        nc.sync.dma_start(base2[:, :, 9:18], t["basex"].ap())

        mv = pixT[:, :, 32:41]

        # fpos = floor(dv + base), robust to trunc-or-round f32->int casts
        nc.vector.tensor_add(typ[:], pixT[:, :, 0:18], base2[:])
        nc.vector.tensor_copy(iy[:], typ[:])
        nc.vector.tensor_copy(fyx[:], iy[:])
        nc.vector.tensor_tensor(cr[:], fyx[:], typ[:], AX.is_gt)
        nc.vector.tensor_sub(fyx[:], fyx[:], cr[:])
        nc.vector.tensor_sub(wyx[:], typ[:], fyx[:])
        fy = fyx[:, :, 0:9]
        fx = fyx[:, :, 9:18]
        wy = wyx[:, :, 0:9]
        wx = wyx[:, :, 9:18]

        # gather indices first, so the main-loop gathers can start while the
        # coefficient planes are still being built.
        CONST = PAD * WP + PAD
        idxt = prol.tile([128, K, 32], FP32, tag="idxt")
        iv = idxt[:].rearrange("p q t -> p t q")
        nc.vector.scalar_tensor_tensor(
            iv[:], fy, float(WP), fx, AX.mult, AX.add
        )
        nc.vector.tensor_scalar_add(iv[:], iv[:], float(CONST))
        nc.vector.tensor_scalar(
            idxt[:], idxt[:], 0.0, float(NROW - 2), AX.max, AX.min
        )
        idxi = prol.tile([128, K, 32], I16, tag="idxi")
        nc.vector.tensor_copy(idxi[:], idxt[:])

        # wrap to 16-partition layout: dst[b, q, 32a + t] = src[16a + b, q, t]
        for a in range(8):
            eng = (nc.sync, nc.scalar)[a % 2]
            eng.dma_start(
                widx[0:16, :, 32 * a : 32 * a + 32],
                idxi[16 * a : 16 * a + 16, :, :],
            )
        # replicate chunk-0/1 columns first (log-doubling) so the early
        # gathers can start; the rest follows on the Act queue (needed only
        # from chunk 2, ~120us later).
        for st in range(3):
            w = 16 << st
            nc.sync.dma_start(widx[w : 2 * w, :, 0:128], widx[0:w, :, 0:128])
        for st in range(3):
            w = 16 << st
            nc.scalar.dma_start(widx[w : 2 * w, :, 128:256], widx[0:w, :, 128:256])

        nc.vector.tensor_mul(mwy1[:], mv, wy)
        nc.vector.tensor_sub(mwy0[:], mv, mwy1[:])

        # planes into [128, q(36), t(32)] f32; q = (2*xc + yc)*9 + k
        coefq = prol.tile([128, Q, 32], FP32, tag="coefq")
        cv = coefq[:].rearrange("p q t -> p t q")
        nc.vector.tensor_mul(cv[:, :, 18:27], mwy0[:], wx)  # P(1,0)
        nc.vector.tensor_sub(cv[:, :, 0:9], mwy0[:], cv[:, :, 18:27])  # P(0,0)
        nc.vector.tensor_mul(cv[:, :, 27:36], mwy1[:], wx)  # P(1,1)
        nc.vector.tensor_sub(cv[:, :, 9:18], mwy1[:], cv[:, :, 27:36])  # P(0,1)

        # plane rows in wrapped-j order: plrow[q, 512a + 16t + b] = plane(q, p)
        plv = plrow[:].rearrange("q (a t b) -> q a t b", a=8, t=32)
        for tcol in range(32):
            stage2 = stgp.tile([128, Q], FP32, tag="tr2_stage", name=f"s2{tcol}")
            nc.vector.tensor_copy(stage2[:], coefq[:, :, tcol])
            ps = trps.tile([Q, 128], FP32, tag="tr2_ps")
            nc.tensor.transpose(ps[:], stage2[:], ident[:, :])
            eng = nc.vector.tensor_copy if tcol % 2 else nc.scalar.copy
            eng(
                plv[:, :, tcol, :],
                ps[:].rearrange("q (a b) -> q a b", a=8),
            )

    # ================= main loop =================
    gp = ctx.enter_context(tc.tile_pool(name="gth", bufs=3))
    ap_pool = ctx.enter_context(tc.tile_pool(name="amul", bufs=3))
    pr_pool = ctx.enter_context(tc.tile_pool(name="prep", bufs=3))
    sp = ctx.enter_context(tc.tile_pool(name="sums", bufs=2))
    op = ctx.enter_context(tc.tile_pool(name="outp", bufs=2))
    gps = ctx.enter_context(tc.tile_pool(name="gemm_ps", bufs=1, space="PSUM"))
    bps = ctx.enter_context(tc.tile_pool(name="brd_ps", bufs=2, space="PSUM"))

    xg_in = dataclasses.replace(
        t["xg2"].ap(), ap=[[2 * C, NROW - 1], [1, 2 * 2 * C]]
    )  # overlapping row pairs, elem = 4 corners x 256ch
    wdef_v = wdef_sb[:].rearrange("p (k c f) -> p k c f", k=K, c=2)

    chunks = [(0, 1024), (1024, 1024), (2048, 1024), (3072, 1024)]
    units = [(ci_, k) for ci_ in range(len(chunks)) for k in range(K)]
    gtiles = {}
    PF = 2

    def emit_gather(u):
        ci_, k = units[u]
        col0, width = chunks[ci_]
        g = gp.tile([128, 8, width], BF16, tag=f"g{width}", name=f"g{u}")
        nc.gpsimd.dma_gather(
            g[:],
            xg_in,
            widx[:, k, col0 // 16 : (col0 + width) // 16],
            num_idxs=width,
            num_idxs_reg=width,
            elem_size=2 * 2 * C,
            elem_step=2 * C,
            transpose=True,
            single_packet=False,
        )
        gtiles[u] = g

    ps_out = {}
    for u in range(len(units) + PF):
        if u < len(units):
            emit_gather(u)
        v = u - PF
        if v < 0:
            continue
        ci_, k = units[v]
        col0, width = chunks[ci_]
        ch = ci_
        if k == 0:
            ps_out[ch] = [
                gps.tile([128, CHUNK], FP32, tag=f"ops{m}", name=f"ops{ch}_{m}")
                for m in range(2)
            ]
        g = gtiles.pop(v)
        a = ap_pool.tile([128, 2, 2, 2, CHUNK], BF16, tag="am", name=f"am{v}")
        for xc in range(2):
            prep = pr_pool.tile(
                [128, 2, CHUNK], BF16, tag="prep", name=f"pr{v}_{xc}"
            )
            for yc in range(2):
                q = (2 * xc + yc) * K + k
                brd = bps.tile([128, CHUNK], FP32, tag="brd", name=f"brd{v}_{xc}{yc}")
                for n0 in range(0, width, 512):
                    nc.tensor.matmul(
                        brd[:, n0 : n0 + 512],
                        sel[:, 128 * q : 128 * q + 128],
                        plrow[:, col0 + n0 : col0 + n0 + 512],
                        start=True,
                        stop=True,
                    )
                nc.scalar.copy(prep[:, yc, :width], brd[:, :width])
            pr_ap = prep[:]
            pr_b = dataclasses.replace(
                pr_ap,
                ap=[list(pr_ap.ap[0]), [CHUNK, 2], [0, 2], [1, width]],
            )
            nc.vector.tensor_tensor(
                a[:, xc, :, :, :width],
                g[:, 4 * xc : 4 * xc + 4, :width].rearrange(
                    "p (y c) j -> p y c j", y=2
                ),
                pr_b,
                AX.mult,
            )
        # single merged add over yc: s[xc, ci, j] = sum_yc am[xc, yc, ci, j]
        s01 = sp.tile([128, 2, 2, CHUNK], BF16, tag="s01", name=f"s01_{v}")
        nc.vector.tensor_add(
            s01[:, :, :, :width], a[:, :, 0, :, :width], a[:, :, 1, :, :width]
        )

        first = k == 0
        last = k == K - 1
        for m in range(2):
            for si in range(2):
                for ci in range(2):
                    for n0 in range(0, width, 512):
                        nc.tensor.matmul(
                            ps_out[ch][m][:, n0 : n0 + 512],
                            wdef_v[:, k, ci, bass.ts(m, 128)],
                            s01[:, si, ci, n0 : n0 + 512],
                            start=(first and si == 0 and ci == 0),
                            stop=(last and si == 1 and ci == 1),
                        )
        if k == K - 1:
            for m in range(2):
                ot = op.tile([128, CHUNK], FP32, tag="ot", name=f"ot{ch}_{m}")
                nc.scalar.copy(ot[:, :width], ps_out[ch][m][:, :width])
                nc.sync.dma_start(
                    t["out"].ap()[bass.ts(m, 128), col0 : col0 + width],
                    ot[:, :width],
                )
            ps_out.pop(ch)


_CACHE = {}


def _get_nc():
    if "nc" not in _CACHE:
        nc = bacc.Bacc("TRN2", target_bir_lowering=False, num_devices=NCORES)
        t = declare_inputs(nc)
        with tile.TileContext(nc) as tc:
            with ExitStack() as ctx:
                build(nc, tc, ctx, t)
        nc.finalize()
        _CACHE["nc"] = nc
    return _CACHE["nc"]


def kernel(x, w_offset, w_mask, w_deform):
    """Full-batch deformable conv. x: [8,256,64,64] f32 -> [8,256,64,64] f32."""
    x = np.asarray(x, dtype=np.float32)
    w_offset = np.asarray(w_offset, dtype=np.float32)
    w_mask = np.asarray(w_mask, dtype=np.float32)
    w_deform = np.asarray(w_deform, dtype=np.float32)
    B = x.shape[0]
    assert B == NCORES
    nc = _get_nc()
    in_maps = [host_inputs(x[b], w_offset, w_mask, w_deform) for b in range(B)]
    res = run_bass_kernel_spmd(nc, in_maps, list(range(NCORES)))
    out = np.empty((B, F, H, W), np.float32)
    for b in range(B):
        o = res.results[b]["out"].reshape(F, 8, 32, 16)  # (a, t, b)
        out[b] = o.transpose(0, 2, 1, 3).reshape(F, H, W)
    return out


# revision 7
# speedup vs baseline: 1.0454x; 1.0454x over previous
"""DeformableConv2D (B=8, C=F=256, H=W=64, K=3x3) on 8 Trainium2 NeuronCores.

Sharding: data-parallel over batch - each of the 8 cores processes one sample.

Per-core pipeline (v7):
  1. offset/mask 3x3 SAME convs as shifted bf16 matmuls (f32 PSUM), output
     rows [dy(9) | dx(9) | pad | mask(9)@32]; conv split in two halves with
     the pixel-partition PE transposes interleaved into the second half.
  2. Bilinear pipeline in pixel-partition f32: fused floor/frac over the
     joint dy|dx block, mask sigmoid after the transpose, gather indices
     wrapped to the 16-partition dma_gather layout (log-doubling replicate).
  3. 36 corner-product planes q=(2*xc+yc)*9+k kept as PE-transposed
     wrapped-j rows (plrow[36, 4096] bf16).
  4. Main loop over (chunk=1024 px, tap): ONE dma_gather per unit fetches
     all 4 bilinear corners x 256 ch in a single 2KB elem (xg2 row-pair
     layout); this keeps the GPSIMD engine (the bottleneck) at its floor.
  5. Corner multiplies on DVE against planes replicated across partitions
     by selector-matmul PE broadcasts (PSUM -> bf16 via Act copies); one
     merged yc-add; the xc-sum is folded into the GEMM (x2 contraction).
  6. bf16 GEMM into f32 PSUM [128, 1024] x2; output written in wrapped-j
     column order, host reorders.

kernel(**inputs) takes the FULL batch and returns the FULL [8,256,64,64] f32
output.
"""

import dataclasses
from contextlib import ExitStack

import numpy as np

import concourse.bass as bass
import concourse.bacc as bacc
import concourse.tile as tile
from concourse import mybir
from concourse.bass_utils import run_bass_kernel_spmd

H = W = 64
HW = H * W
C = 256
F = 256
K = 9
OC = 41  # conv out rows: 0-8 dy, 9-17 dx, 32-40 mask
PAD = 8
HP = H + 2 * PAD  # 80
WP = W + 2 * PAD  # 80
NROW = HP * WP  # 6400
H1 = H + 2  # 66
W1 = W + 2
HW1 = H1 * W1  # 4356
MARG = 68

FP32 = mybir.dt.float32
I32 = mybir.dt.int32
BF16 = mybir.dt.bfloat16
I16 = mybir.dt.int16
AX = mybir.AluOpType
AF = mybir.ActivationFunctionType

CHUNK = 1024
NCHUNK = HW // CHUNK  # 4
Q = 4 * K  # 36 planes
NCORES = 8


def host_inputs(x, w_offset, w_mask, w_deform):
    """Per-sample layout prep. x: [C,H,W] float32 one sample."""
    import ml_dtypes

    ins = {}
    xp1 = np.zeros((C, H1, W1), ml_dtypes.bfloat16)
    xp1[:, 1:-1, 1:-1] = x
    ins["xpad1"] = np.ascontiguousarray(xp1.reshape(C, HW1))

    # xg2 row (y, x) = [xpad[y, x, :], xpad[y+1, x, :]]  (bf16)
    xp2 = np.zeros((HP + 1, WP, C), ml_dtypes.bfloat16)
    xp2[PAD : PAD + H, PAD : PAD + W, :] = np.transpose(x, (1, 2, 0)).astype(
        ml_dtypes.bfloat16
    )
    xg2 = np.concatenate([xp2[:-1], xp2[1:]], axis=2)  # [HP, WP, 2C]
    ins["xg2"] = np.ascontiguousarray(xg2.reshape(NROW, 2 * C))

    # conv weights, out-channel order [dy(9) | dx(9) | pad | mask(9) at 32]
    wt = np.zeros((3, 3, C, OC), np.float32)
    wo = np.transpose(w_offset, (2, 3, 1, 0))  # [3,3,C,18]
    wt[:, :, :, 0:9] = wo[:, :, :, 0::2]  # dy_k = offset channel 2k
    wt[:, :, :, 9:18] = wo[:, :, :, 1::2]  # dx_k = offset channel 2k+1
    wt[:, :, :, 32:41] = np.transpose(w_mask, (2, 3, 1, 0))
    ins["wconv"] = np.ascontiguousarray(
        wt.reshape(K, 2, 128, OC), dtype=ml_dtypes.bfloat16
    )

    wd = np.transpose(w_deform.reshape(F, C, K), (2, 1, 0))  # [k, c, f]
    ins["wdef"] = np.ascontiguousarray(
        wd.reshape(K, 2, 128, F).astype(ml_dtypes.bfloat16)
    )

    p = np.arange(HW)
    hh = (p // W).astype(np.float32)
    ww = (p % W).astype(np.float32)
    ky = np.repeat(np.arange(3) - 1, 3).astype(np.float32)
    kx = np.tile(np.arange(3) - 1, 3).astype(np.float32)
    basey = (hh[:, None] + ky[None, :]).reshape(32, 128, K).transpose(1, 0, 2)
    basex = (ww[:, None] + kx[None, :]).reshape(32, 128, K).transpose(1, 0, 2)
    ins["basey"] = np.ascontiguousarray(basey, dtype=np.float32)
    ins["basex"] = np.ascontiguousarray(basex, dtype=np.float32)
    ins["ident"] = np.eye(128, dtype=np.float32)
    ins["ones2"] = np.ones((128, 2), np.float32)
    # sel[p, q, :] = (p == q): stationary that broadcasts plrow row q to all
    # 128 PSUM partitions
    sel = np.zeros((Q, Q, 128), ml_dtypes.bfloat16)
    for q in range(Q):
        sel[q, q, :] = 1.0
    ins["sel"] = sel.reshape(Q, Q * 128)
    return ins


def declare_inputs(nc):
    t = {}
    t["xpad1"] = nc.dram_tensor("xpad1", [C, HW1], BF16, kind="ExternalInput")
    t["xg2"] = nc.dram_tensor("xg2", [NROW, 2 * C], BF16, kind="ExternalInput")
    t["wconv"] = nc.dram_tensor("wconv", [K, 2, 128, OC], BF16, kind="ExternalInput")
    t["wdef"] = nc.dram_tensor("wdef", [K, 2, 128, F], BF16, kind="ExternalInput")
    t["basey"] = nc.dram_tensor("basey", [128, 32, K], FP32, kind="ExternalInput")
    t["basex"] = nc.dram_tensor("basex", [128, 32, K], FP32, kind="ExternalInput")
    t["ident"] = nc.dram_tensor("ident", [128, 128], FP32, kind="ExternalInput")
    t["ones2"] = nc.dram_tensor("ones2", [128, 2], FP32, kind="ExternalInput")
    t["sel"] = nc.dram_tensor("sel", [Q, Q * 128], BF16, kind="ExternalInput")
    # columns in wrapped-j order: j = 16*(32a + t) + b <-> pixel 128t + 16a + b
    t["out"] = nc.dram_tensor("out", [F, HW], FP32, kind="ExternalOutput")
    return t


def build(nc, tc, ctx: ExitStack, t):
    keep = ctx.enter_context(tc.tile_pool(name="keep", bufs=1))

    ident = keep.tile([128, 128], FP32)
    ones2 = keep.tile([128, 2], FP32)
    sel = keep.tile([Q, Q * 128], BF16)
    wdef_sb = keep.tile([128, K * 2 * F], BF16)
    widx = keep.tile([128, K, HW // 16], I16)
    plrow = keep.tile([Q, HW], BF16)  # wrapped-j order plane rows

    def load_aux():
        # off the critical path; issued on Act/DVE DMA queues
        nc.scalar.dma_start(ident[:], t["ident"].ap())
        nc.scalar.dma_start(ones2[:], t["ones2"].ap())
        nc.scalar.dma_start(sel[:], t["sel"].ap())
        nc.scalar.dma_start(
            wdef_sb[:].rearrange("p (k c f) -> p k c f", k=K, c=2),
            t["wdef"].ap().rearrange("k c p f -> p k c f"),
        )

    # ================= prologue =================
    with tc.tile_pool(name="prol", bufs=1) as prol, tc.tile_pool(
        name="stgp", bufs=4
    ) as stgp, tc.tile_pool(
        name="prps", bufs=2, space="PSUM"
    ) as prps, tc.tile_pool(name="trps", bufs=3, space="PSUM") as trps:
        wconv_sb = prol.tile([128, K * 2 * OC], BF16, tag="wconv")
        nc.sync.dma_start(
            wconv_sb[:].rearrange("p (k c o) -> p k c o", k=K, c=2),
            t["wconv"].ap().rearrange("k c p o -> p k c o"),
        )
        xp1 = [
            prol.tile([128, HW1 + 2 * MARG], BF16, tag=f"xp1_{i}", name=f"xp1_{i}")
            for i in range(2)
        ]
        for i in range(2):
            nc.vector.memset(xp1[i][:, 0:MARG], 0.0)
            nc.vector.memset(xp1[i][:, MARG + HW1 :], 0.0)
            nc.sync.dma_start(
                xp1[i][:, MARG : MARG + HW1], t["xpad1"].ap()[bass.ts(i, 128), :]
            )
        load_aux()

        # conv into two half tiles; pixT transposes interleaved with the
        # second half's matmuls so they overlap on all engines.
        # A: rows 0..39 (tcols 0..18); B: rows 40..65
        JSPLIT = 40 * W1  # 2640
        convA = prol.tile([OC, JSPLIT], FP32, tag="convA")
        convB = prol.tile([OC, HW1 - JSPLIT], FP32, tag="convB")
        NCONV = 4 * W1  # 264 (4 rows, 1 PSUM bank)
        wviews = wconv_sb[:].rearrange("p (k c o) -> p k c o", k=K, c=2)
        pixT = prol.tile([128, 32, OC], FP32, tag="pixT")

        def conv_row(h):  # [OC, W1] view of conv output row h
            if (h + 1) * W1 <= JSPLIT:
                return convA[:, h * W1 : (h + 1) * W1]
            return convB[:, h * W1 - JSPLIT : (h + 1) * W1 - JSPLIT]

        def emit_transpose(tcol):
            h0 = 2 * tcol
            stage = stgp.tile([OC, 128], FP32, tag="tr_stage", name=f"st{tcol}")
            for r in range(2):
                nc.vector.tensor_copy(
                    stage[:, 64 * r : 64 * r + 64],
                    conv_row(h0 + 1 + r)[:, 1 : 1 + W],
                )
            ps = trps.tile([128, OC], FP32, tag="tr_ps")
            nc.tensor.transpose(ps[:], stage[:], ident[:OC, :OC])
            if tcol % 2:
                nc.vector.tensor_copy(pixT[:, tcol, :], ps[:])
            else:
                nc.scalar.copy(pixT[:, tcol, :], ps[:])

        def emit_conv_block(j0):
            n = min(NCONV, HW1 - j0)
            ps = prps.tile([OC, NCONV], FP32, tag="conv_ps")
            first = True
            for ci in range(2):
                for k in range(K):
                    off = (k // 3 - 1) * W1 + (k % 3 - 1)
                    nc.tensor.matmul(
                        ps[:, :n],
                        wviews[:, k, ci, :],
                        xp1[ci][:, MARG + j0 + off : MARG + j0 + off + n],
                        start=first,
                        stop=(ci == 1 and k == K - 1),
                    )
                    first = False
            if j0 < JSPLIT:
                nc.scalar.copy(convA[:, j0 : j0 + n], ps[:, :n])
            else:
                nc.scalar.copy(convB[:, j0 - JSPLIT : j0 - JSPLIT + n], ps[:, :n])

        for j0 in range(0, JSPLIT, NCONV):
            emit_conv_block(j0)
        # second half: interleave pixT transposes as their rows become ready
        # (tcol T reads rows 2T+1, 2T+2; A covers rows < 40)
        pend = list(range(32))
        for j0 in range(JSPLIT, HW1, NCONV):
            emit_conv_block(j0)
            rows_done = min((j0 + NCONV), HW1) // W1
            while pend and 2 * pend[0] + 2 < rows_done:
                emit_transpose(pend.pop(0))
        for tcol in pend:
            emit_transpose(tcol)
        nc.scalar.activation(
            pixT[:, :, 32:41], pixT[:, :, 32:41], AF.Sigmoid
        )

        # ---- coefficient pipeline (f32, pixel-partition) ----
        # dy|dx processed together: pixT cols 0:18 match base2 = [basey|basex]
        def pt2(tag):
            return prol.tile([128, 32, 2 * K], FP32, tag=tag, name=tag)

        typ = pt2("typ")
        fyx = pt2("fyx")
        wyx = pt2("wyx")
        cr = pt2("cr")
        mwy0 = prol.tile([128, 32, K], FP32, tag="mwy0", name="mwy0")
        mwy1 = prol.tile([128, 32, K], FP32, tag="mwy1", name="mwy1")
        iy = prol.tile([128, 32, 2 * K], I32, tag="iy")
        base2 = prol.tile([128, 32, 2 * K], FP32, tag="base2")
        nc.sync.dma_start(base2[:, :, 0:9], t["basey"].ap())
        nc.sync.dma_start(base2[:, :, 9:18], t["basex"].ap())

        mv = pixT[:, :, 32:41]

        # fpos = floor(dv + base), robust to trunc-or-round f32->int casts
        nc.vector.tensor_add(typ[:], pixT[:, :, 0:18], base2[:])
        nc.vector.tensor_copy(iy[:], typ[:])
        nc.vector.tensor_copy(fyx[:], iy[:])
        nc.vector.tensor_tensor(cr[:], fyx[:], typ[:], AX.is_gt)
        nc.vector.tensor_sub(fyx[:], fyx[:], cr[:])
        nc.vector.tensor_sub(wyx[:], typ[:], fyx[:])
        fy = fyx[:, :, 0:9]
        fx = fyx[:, :, 9:18]
        wy = wyx[:, :, 0:9]
        wx = wyx[:, :, 9:18]

        # gather indices first, so the main-loop gathers can start while the
        # coefficient planes are still being built.
        CONST = PAD * WP + PAD
        idxt = prol.tile([128, K, 32], FP32, tag="idxt")
        iv = idxt[:].rearrange("p q t -> p t q")
        nc.vector.scalar_tensor_tensor(
            iv[:], fy, float(WP), fx, AX.mult, AX.add
        )
        nc.vector.tensor_scalar_add(iv[:], iv[:], float(CONST))
        nc.vector.tensor_scalar(
            idxt[:], idxt[:], 0.0, float(NROW - 2), AX.max, AX.min
        )
        idxi = prol.tile([128, K, 32], I16, tag="idxi")
        nc.vector.tensor_copy(idxi[:], idxt[:])

        # wrap to 16-partition layout: dst[b, q, 32a + t] = src[16a + b, q, t]
        for a in range(8):
            eng = (nc.sync, nc.scalar)[a % 2]
            eng.dma_start(
                widx[0:16, :, 32 * a : 32 * a + 32],
                idxi[16 * a : 16 * a + 16, :, :],
            )
        # replicate chunk-0/1 columns first (log-doubling) so the early
        # gathers can start; the rest follows on the Act queue (needed only
        # from chunk 2, ~120us later).
        for st in range(3):
            w = 16 << st
            nc.sync.dma_start(widx[w : 2 * w, :, 0:128], widx[0:w, :, 0:128])
        for st in range(3):
            w = 16 << st
            nc.scalar.dma_start(widx[w : 2 * w, :, 128:256], widx[0:w, :, 128:256])

        nc.vector.tensor_mul(mwy1[:], mv, wy)
        nc.vector.tensor_sub(mwy0[:], mv, mwy1[:])

        # planes into [128, q(36), t(32)] f32; q = (2*xc + yc)*9 + k
        coefq = prol.tile([128, Q, 32], FP32, tag="coefq")
        cv = coefq[:].rearrange("p q t -> p t q")
        nc.vector.tensor_mul(cv[:, :, 18:27], mwy0[:], wx)  # P(1,0)
        nc.vector.tensor_sub(cv[:, :, 0:9], mwy0[:], cv[:, :, 18:27])  # P(0,0)
        nc.vector.tensor_mul(cv[:, :, 27:36], mwy1[:], wx)  # P(1,1)
        nc.vector.tensor_sub(cv[:, :, 9:18], mwy1[:], cv[:, :, 27:36])  # P(0,1)

        # plane rows in wrapped-j order: plrow[q, 512a + 16t + b] = plane(q, p)
        plv = plrow[:].rearrange("q (a t b) -> q a t b", a=8, t=32)
        for tcol in range(32):
            stage2 = stgp.tile([128, Q], FP32, tag="tr2_stage", name=f"s2{tcol}")
            nc.vector.tensor_copy(stage2[:], coefq[:, :, tcol])
            ps = trps.tile([Q, 128], FP32, tag="tr2_ps")
            nc.tensor.transpose(ps[:], stage2[:], ident[:, :])
            eng = nc.vector.tensor_copy if tcol % 2 else nc.scalar.copy
            eng(
                plv[:, :, tcol, :],
                ps[:].rearrange("q (a b) -> q a b", a=8),
            )

    # ================= main loop =================
    gp = ctx.enter_context(tc.tile_pool(name="gth", bufs=3))
    ap_pool = ctx.enter_context(tc.tile_pool(name="amul", bufs=3))
    pr_pool = ctx.enter_context(tc.tile_pool(name="prep", bufs=3))
    sp = ctx.enter_context(tc.tile_pool(name="sums", bufs=2))
    op = ctx.enter_context(tc.tile_pool(name="outp", bufs=2))
    gps = ctx.enter_context(tc.tile_pool(name="gemm_ps", bufs=1, space="PSUM"))
    bps = ctx.enter_context(tc.tile_pool(name="brd_ps", bufs=2, space="PSUM"))

    xg_in = dataclasses.replace(
        t["xg2"].ap(), ap=[[2 * C, NROW - 1], [1, 2 * 2 * C]]
    )  # overlapping row pairs, elem = 4 corners x 256ch
    wdef_v = wdef_sb[:].rearrange("p (k c f) -> p k c f", k=K, c=2)

    chunks = [(0, 1024), (1024, 1024), (2048, 1024), (3072, 1024)]
    units = [(ci_, k) for ci_ in range(len(chunks)) for k in range(K)]
    gtiles = {}
    PF = 2

    def emit_gather(u):
        ci_, k = units[u]
        col0, width = chunks[ci_]
        g = gp.tile([128, 8, width], BF16, tag=f"g{width}", name=f"g{u}")
        nc.gpsimd.dma_gather(
            g[:],
            xg_in,
            widx[:, k, col0 // 16 : (col0 + width) // 16],
            num_idxs=width,
            num_idxs_reg=width,
            elem_size=2 * 2 * C,
            elem_step=2 * C,
            transpose=True,
            single_packet=False,
        )
        gtiles[u] = g

    ps_out = {}
    for u in range(len(units) + PF):
        if u < len(units):
            emit_gather(u)
        v = u - PF
        if v < 0:
            continue
        ci_, k = units[v]
        col0, width = chunks[ci_]
        ch = ci_
        if k == 0:
            ps_out[ch] = [
                gps.tile([128, CHUNK], FP32, tag=f"ops{m}", name=f"ops{ch}_{m}")
                for m in range(2)
            ]
        g = gtiles.pop(v)
        a = ap_pool.tile([128, 2, 2, 2, CHUNK], BF16, tag="am", name=f"am{v}")
        for xc in range(2):
            prep = pr_pool.tile(
                [128, 2, CHUNK], BF16, tag="prep", name=f"pr{v}_{xc}"
            )
            for yc in range(2):
                q = (2 * xc + yc) * K + k
                brd = bps.tile([128, CHUNK], FP32, tag="brd", name=f"brd{v}_{xc}{yc}")
                for n0 in range(0, width, 512):
                    nc.tensor.matmul(
                        brd[:, n0 : n0 + 512],
                        sel[:, 128 * q : 128 * q + 128],
                        plrow[:, col0 + n0 : col0 + n0 + 512],
                        start=True,
                        stop=True,
                    )
                nc.scalar.copy(prep[:, yc, :width], brd[:, :width])
            pr_ap = prep[:]
            pr_b = dataclasses.replace(
                pr_ap,
                ap=[list(pr_ap.ap[0]), [CHUNK, 2], [0, 2], [1, width]],
            )
            nc.vector.tensor_tensor(
                a[:, xc, :, :, :width],
                g[:, 4 * xc : 4 * xc + 4, :width].rearrange(
                    "p (y c) j -> p y c j", y=2
                ),
                pr_b,
                AX.mult,
            )
        # single merged add over yc: s[xc, ci, j] = sum_yc am[xc, yc, ci, j]
        s01 = sp.tile([128, 2, 2, CHUNK], BF16, tag="s01", name=f"s01_{v}")
        nc.vector.tensor_add(
            s01[:, :, :, :width], a[:, :, 0, :, :width], a[:, :, 1, :, :width]
        )

        first = k == 0
        last = k == K - 1
        for m in range(2):
            for si in range(2):
                for ci in range(2):
                    for n0 in range(0, width, 512):
                        nc.tensor.matmul(
                            ps_out[ch][m][:, n0 : n0 + 512],
                            wdef_v[:, k, ci, bass.ts(m, 128)],
                            s01[:, si, ci, n0 : n0 + 512],
                            start=(first and si == 0 and ci == 0),
                            stop=(last and si == 1 and ci == 1),
                        )
        if k == K - 1:
            for m in range(2):
                ot = op.tile([128, CHUNK], FP32, tag="ot", name=f"ot{ch}_{m}")
                nc.scalar.copy(ot[:, :width], ps_out[ch][m][:, :width])
                nc.sync.dma_start(
                    t["out"].ap()[bass.ts(m, 128), col0 : col0 + width],
                    ot[:, :width],
                )
            ps_out.pop(ch)


_CACHE = {}


def _get_nc():
    if "nc" not in _CACHE:
        nc = bacc.Bacc("TRN2", target_bir_lowering=False, num_devices=NCORES)
        t = declare_inputs(nc)
        with tile.TileContext(nc) as tc:
            with ExitStack() as ctx:
                build(nc, tc, ctx, t)
        nc.finalize()
        _CACHE["nc"] = nc
    return _CACHE["nc"]


def kernel(x, w_offset, w_mask, w_deform):
    """Full-batch deformable conv. x: [8,256,64,64] f32 -> [8,256,64,64] f32."""
    x = np.asarray(x, dtype=np.float32)
    w_offset = np.asarray(w_offset, dtype=np.float32)
    w_mask = np.asarray(w_mask, dtype=np.float32)
    w_deform = np.asarray(w_deform, dtype=np.float32)
    B = x.shape[0]
    assert B == NCORES
    nc = _get_nc()
    in_maps = [host_inputs(x[b], w_offset, w_mask, w_deform) for b in range(B)]
    res = run_bass_kernel_spmd(nc, in_maps, list(range(NCORES)))
    out = np.empty((B, F, H, W), np.float32)
    for b in range(B):
        o = res.results[b]["out"].reshape(F, 8, 32, 16)  # (a, t, b)
        out[b] = o.transpose(0, 2, 1, 3).reshape(F, H, W)
    return out
